# revision 1
# baseline (speedup 1.0000x reference)
"""Concordance-index (C-index) kernel for Trainium2, 8 NeuronCores.

Math
----
Reference computes, over all pairs i<j of N=16384 samples:
    cc = ((y_i>=y_j & yh_i>=yh_j & st_j) | (y_i<=y_j & yh_i<=yh_j & st_i)) & triu
    tp = ((y_i<=y_j & st_i) | (y_i>=y_j & st_j)) & triu
    out = sum(cc) / sum(tp)

Key reduction: columns with st_j = 0 contribute nothing to either count
(A1(i,j) = [y_i>=y_j]*[yh_i>=yh_j]*st_j and A2(i,j) = [y_i>=y_j]*st_j both
vanish), so the pairwise sweep is N x ns over (all i) x (event j only):
    sum(cc) = S1 - ns,  S1 = sum_{i, j in E} [y_i>=y_j][yh_i>=yh_j]
    sum(tp) = S2 - ns,  S2 = sum_{i, j in E} [y_i>=y_j],   ns = |E|
(exact up to pairs simultaneously tied in y and yh — absent here).

Sharding: the ns event samples are packed into NCORES*JT_E*128 j-slots
(j on SBUF partitions, JT_E j-tiles per core); unused slots are padded
with y=yh=+BIG, which contributes exactly zero through every formula
below.  i is streamed along the free axis in F=4096 DMA-broadcast tiles.

Per (i-tile it, j-tile jt), col = it*JT_E+jt:
    g = sign(y_i - y_j)     ScalarE Sign + fused row-sum -> acc_sg[col]
    h = sign(yh_i - yh_j)   ScalarE Sign + fused row-sum -> acc_sh[col]
        or (most cols) h01 = [yh_i >= yh_j] on VectorE with fused row-sum
    p = g*h                 VectorE tensor_tensor (2x mode)
    sum of p                TensorE ones-weight matmuls -> PSUM accumulator
                            (acc_ps for sign-h cols, acc_p01 for 01-h cols)
Host reconstructs S1/S2 with exact integer algebra in float64:
    sign-h cells: G*H = (gh + g + h + 1)/4      (diag corr +3/4 per event)
    01-h  cells: G*H = (g*h01 + h01)/2          (diag corr +1/2 per event)
    S2 = (sum_all g + n_tiles*Mt)/2 + ns/2
and mirrors the reference's float32 division.
"""

import math
import os
import sys

import numpy as np

for _p in ("/opt/trn_rl_repo", "/root/.axon_site", "/root/.axon_site/_ro/trn_rl_repo"):
    if os.path.isdir(_p) and _p not in sys.path:
        sys.path.append(_p)

import concourse.bacc as bacc
import concourse.bass as bass
import concourse.mybir as mybir
from concourse import bass_utils
from concourse import tile

N = 16384
P = 128
NCORES = 8
F = 4096                 # i-tile width (free axis)
IT = N // F              # 4 i-tiles
BIG = np.float32(1e30)

FP32 = mybir.dt.float32
BF16 = mybir.dt.bfloat16
Alu = mybir.AluOpType
ActF = mybir.ActivationFunctionType


def _act_h_cols(nt):
    """Columns whose h runs on ScalarE as sign (engine balance)."""
    want = max(1, round(nt * 8 / 36))
    return frozenset([c for c in range(nt) if c % 3 == 0][:want])


def _pe_h_cols(nt):
    """01-h columns whose column-sum goes to TensorE (rest use the fused
    VectorE accumulator, which runs at 1x)."""
    rest = [c for c in range(nt) if c not in _act_h_cols(nt)]
    return frozenset(c for i, c in enumerate(rest) if i % 7 < 5)


def build_bass(jt_e):
    nt = IT * jt_e
    act_h = _act_h_cols(nt)
    pe_h = _pe_h_cols(nt)
    nc = bacc.Bacc(debug=False, num_devices=NCORES)

    y_full = nc.dram_tensor("y_full", [1, N], FP32, kind="ExternalInput")
    yh_full = nc.dram_tensor("yh_full", [1, N], FP32, kind="ExternalInput")
    y_sl = nc.dram_tensor("y_sl", [P, jt_e], FP32, kind="ExternalInput")
    yh_sl = nc.dram_tensor("yh_sl", [P, jt_e], FP32, kind="ExternalInput")
    o_sg = nc.dram_tensor("o_sg", [P, nt], FP32, kind="ExternalOutput")
    o_sh = nc.dram_tensor("o_sh", [P, nt], FP32, kind="ExternalOutput")
    o_ps = nc.dram_tensor("o_ps", [1, 512], FP32, kind="ExternalOutput")
    o_p01 = nc.dram_tensor("o_p01", [1, 512], FP32, kind="ExternalOutput")
    o_h01 = nc.dram_tensor("o_h01", [1, 512], FP32, kind="ExternalOutput")

    n_mm_s = len(act_h) * (F // 512)
    n_mm_01 = (nt - len(act_h)) * (F // 512)
    n_mm_h = len(pe_h) * (F // 512)

    with tile.TileContext(nc) as tc:
        with (
            tc.tile_pool(name="const", bufs=1) as cpool,
            tc.tile_pool(name="bcast", bufs=2) as bpool,
            tc.tile_pool(name="work", bufs=5) as wpool,
            tc.tile_pool(name="psum", bufs=1, space="PSUM") as ppool,
        ):
            y_j = cpool.tile([P, jt_e], FP32)
            nc.sync.dma_start(out=y_j[:, :], in_=y_sl[:, :])
            yh_j = cpool.tile([P, jt_e], FP32)
            nc.sync.dma_start(out=yh_j[:, :], in_=yh_sl[:, :])
            neg_y = cpool.tile([P, jt_e], FP32)
            nc.vector.tensor_scalar_mul(neg_y[:, :], y_j[:, :], -1.0)
            neg_yh = cpool.tile([P, jt_e], FP32)
            nc.vector.tensor_scalar_mul(neg_yh[:, :], yh_j[:, :], -1.0)

            ones_w = cpool.tile([P, 1], BF16)
            nc.vector.memset(ones_w[:, :], 1.0)

            acc_sg = cpool.tile([P, nt], FP32)
            acc_sh = cpool.tile([P, nt], FP32)
            nc.vector.memset(acc_sh[:, :], 0.0)
            acc_ps = ppool.tile([1, 512], FP32)
            acc_p01 = ppool.tile([1, 512], FP32)
            acc_h01 = ppool.tile([1, 512], FP32)
            seen = {"ps": 0, "p01": 0, "h01": 0}
            n_mm = {"ps": n_mm_s, "p01": n_mm_01, "h01": n_mm_h}

            def pe_reduce(key, acc, src):
                for ch in range(F // 512):
                    seen[key] += 1
                    nc.tensor.matmul(
                        acc[0:1, 0:512],
                        ones_w[:, :],
                        src[:, ch * 512:(ch + 1) * 512],
                        start=(seen[key] == 1),
                        stop=(seen[key] == n_mm[key]),
                    )

            for it in range(IT):
                yib = bpool.tile([P, F], FP32, tag="yib")
                nc.sync.dma_start(
                    out=yib[:, :],
                    in_=y_full[0:1, it * F:(it + 1) * F].to_broadcast((P, F)),
                )
                yhib = bpool.tile([P, F], FP32, tag="yhib")
                nc.sync.dma_start(
                    out=yhib[:, :],
                    in_=yh_full[0:1, it * F:(it + 1) * F].to_broadcast((P, F)),
                )
                for jt in range(jt_e):
                    col = it * jt_e + jt
                    g = wpool.tile([P, F], BF16, tag="g")
                    nc.scalar.activation(
                        out=g[:, :], in_=yib[:, :], func=ActF.Sign,
                        bias=neg_y[:, jt:jt + 1], scale=1.0,
                        accum_out=acc_sg[:, col:col + 1],
                    )
                    h = wpool.tile([P, F], BF16, tag="h")
                    if col in act_h:
                        nc.scalar.activation(
                            out=h[:, :], in_=yhib[:, :], func=ActF.Sign,
                            bias=neg_yh[:, jt:jt + 1], scale=1.0,
                            accum_out=acc_sh[:, col:col + 1],
                        )
                    elif col in pe_h:
                        # plain 2x compare; column-sum via TensorE
                        nc.vector.tensor_scalar(
                            out=h[:, :], in0=yhib[:, :],
                            scalar1=yh_j[:, jt:jt + 1], scalar2=None,
                            op0=Alu.is_ge,
                        )
                        pe_reduce("h01", acc_h01, h)
                    else:
                        # accum mode: out = in0 op0 s1; accum = sum(out) op1 s2
                        nc.vector.tensor_scalar(
                            out=h[:, :], in0=yhib[:, :],
                            scalar1=yh_j[:, jt:jt + 1], scalar2=0.0,
                            op0=Alu.is_ge, op1=Alu.add,
                            accum_out=acc_sh[:, col:col + 1],
                        )
                    p = wpool.tile([P, F], BF16, tag="p")
                    nc.vector.tensor_tensor(
                        out=p[:, :], in0=g[:, :], in1=h[:, :], op=Alu.mult)
                    pe_reduce("ps" if col in act_h else "p01",
                              acc_ps if col in act_h else acc_p01, p)

            nc.sync.dma_start(out=o_sg[:, :], in_=acc_sg[:, :])
            nc.sync.dma_start(out=o_sh[:, :], in_=acc_sh[:, :])
            for acc, o in ((acc_ps, o_ps), (acc_p01, o_p01), (acc_h01, o_h01)):
                stg = cpool.tile([1, 512], FP32, tag=f"stg_{o.name}")
                nc.vector.tensor_copy(out=stg[:, :], in_=acc[0:1, 0:512])
                nc.sync.dma_start(out=o[:, :], in_=stg[:, :])

    nc.compile()
    return nc


_NC_CACHE = {}


def _get_nc(jt_e):
    if jt_e not in _NC_CACHE:
        _NC_CACHE[jt_e] = build_bass(jt_e)
    return _NC_CACHE[jt_e]


def _shard(y, yh, status):
    """Pack event samples into j-slots; pad with +BIG (zero contribution)."""
    ev = np.nonzero(status == 1)[0]
    ns = len(ev)
    jt_e = max(1, math.ceil(ns / (NCORES * P)))
    slots = NCORES * jt_e * P
    y_e = np.full(slots, BIG, dtype=np.float32)
    yh_e = np.full(slots, BIG, dtype=np.float32)
    y_e[:ns] = y[ev]
    yh_e[:ns] = yh[ev]
    return ev, jt_e, y_e, yh_e


def make_in_maps(y, y_hat, status, shard):
    y = np.ascontiguousarray(np.asarray(y, dtype=np.float32))
    yh = np.ascontiguousarray(np.asarray(y_hat, dtype=np.float32))
    ev, jt_e, y_e, yh_e = shard
    y2 = y.reshape(1, N)
    yh2 = yh.reshape(1, N)
    per = jt_e * P
    in_maps = []
    for c in range(NCORES):
        sl = slice(c * per, (c + 1) * per)
        in_maps.append({
            "y_full": y2,
            "yh_full": yh2,
            # slot s = c*per + t*P + p  ->  [p, t]
            "y_sl": np.ascontiguousarray(y_e[sl].reshape(jt_e, P).T),
            "yh_sl": np.ascontiguousarray(yh_e[sl].reshape(jt_e, P).T),
        })
    return in_maps


def combine(results, status, shard):
    """Exact integer algebra (float64) over device partial sums."""
    ev, jt_e, y_e, yh_e = shard
    ns = float(len(ev))
    nt = IT * jt_e
    act_h = _act_h_cols(nt)
    Mt = float(P) * float(F)
    per = jt_e * P
    S1 = 0.0
    S2 = 0.0
    for c, r in enumerate(results):
        sg = r["o_sg"].astype(np.float64)
        sh = r["o_sh"].astype(np.float64)
        A_s = float(r["o_ps"].astype(np.float64).sum())
        A_01 = float(r["o_p01"].astype(np.float64).sum())
        s_cols = sorted(act_h)
        o_cols = [x for x in range(nt) if x not in act_h]
        B_s = float(sg[:, s_cols].sum())
        C_s = float(sh[:, s_cols].sum())
        # 01-column h sums: PE accumulator for pe_h cols, fused DVE
        # accumulator (o_sh columns) for the rest
        C_01 = float(r["o_h01"].astype(np.float64).sum())
        C_01 += float(sh[:, [x for x in o_cols if x not in _pe_h_cols(nt)]].sum())
        S1 += (A_s + B_s + C_s + len(s_cols) * Mt) / 4.0
        S1 += (A_01 + C_01) / 2.0
        S2 += (float(sg.sum()) + nt * Mt) / 2.0
    # diagonal corrections: event e in slot s pairs with itself at
    # i-tile it_e = ev[s]//F, j-tile jt = (s % per)//P of core s//per.
    for s, orig in enumerate(ev):
        jt_e_local = (s % per) // P
        col = (orig // F) * jt_e + jt_e_local
        S1 += 0.75 if col in act_h else 0.5
    S2 += ns / 2.0
    c32 = np.float32(S1 - ns)
    t32 = np.float32(S2 - ns)
    return np.asarray(np.float32(c32 / t32))


def kernel(y, y_hat, status, _run_kwargs=None):
    status = np.asarray(status)
    shard = _shard(np.asarray(y), np.asarray(y_hat), status)
    nc = _get_nc(shard[1])
    in_maps = make_in_maps(y, y_hat, status, shard)
    kw = dict(_run_kwargs or {})
    res = bass_utils.run_bass_kernel_spmd(
        nc, in_maps, core_ids=list(range(NCORES)), **kw)
    out = combine(res.results, status, shard)
    if _run_kwargs is not None:
        return out, res
    return out


if __name__ == "__main__":
    rng = np.random.default_rng(0)
    y = rng.standard_normal(N).astype(np.float32)
    yh = rng.standard_normal(N).astype(np.float32)
    st = (rng.integers(0, 2, N)).astype(np.int32)
    print(kernel(y, yh, st))

